# revision 1
# baseline (speedup 1.0000x reference)
"""Trainium2 Bass kernel for nn_Encoder_base (5x ChebConv GNN + pool + MLP).

Distribution over 8 NeuronCores:
  - level-0 ChebConv props: edge-sharded by destination (all 32 batches,
    96 = 32*3 features), selection-matmul scatter + 2 DRAM AllGathers
  - levels 1-3: batch-sharded (4 batches/core, 128 = 4*32 features);
    level-1 props sparse (indirect-DMA row gathers + selection matmuls),
    levels 2-3 dense-S matmuls
  - channel mixes as block-diagonal matmuls in feature-major layout
  - MLP: output-feature sharded (each core owns 512 cols of W6/W7/W8 and
    512 rows of W9), BatchNorm is local per feature; activations AllGathered
"""
import numpy as np
import concourse.bass as bass
import concourse.bacc as bacc
import concourse.tile as tile
from concourse import mybir, bass_utils
from concourse.masks import make_identity

F32 = mybir.dt.float32
I32 = mybir.dt.int32
I16 = mybir.dt.int16
AF = mybir.ActivationFunctionType
ALU = mybir.AluOpType
AX = mybir.AxisListType
RG = [list(range(8))]
NCORES = 8
N0, N1, N2, N3 = 16384, 4096, 1024, 128
EPS = 1e-5

_CACHE = {}


# ---------------------------------------------------------------- host prep
def _prep_prop(row, col, we, n_dest, n_shard):
    """Sorted-by-dest edges -> 128-dest windows, 128-edge chunks, padded so
    chunk counts per window match across shards (one SPMD program)."""
    window = 128
    order = np.argsort(row, kind="stable")
    row, col, we = row[order], col[order], we[order]
    per = n_dest // n_shard
    nwin = per // window
    counts = np.zeros((n_shard, nwin), np.int64)
    lists = {}
    for s in range(n_shard):
        lo = s * per
        for wi in range(nwin):
            wlo = lo + wi * window
            a = np.searchsorted(row, wlo, side="left")
            b = np.searchsorted(row, wlo + window, side="left")
            lists[(s, wi)] = (row[a:b] - wlo, col[a:b], we[a:b])
            counts[s, wi] = (b - a + 127) // 128
    ncw = np.maximum(counts.max(axis=0), 1)
    C = int(ncw.sum())
    src = np.zeros((n_shard, C, 128), np.int32)
    dst = np.full((n_shard, C, 128), 200.0, np.float32)
    wea = np.zeros((n_shard, C, 128), np.float32)
    for s in range(n_shard):
        base = 0
        for wi in range(nwin):
            dl, cl, wl = lists[(s, wi)]
            n = len(dl)
            k = int(ncw[wi])
            src[s, base:base + k].reshape(-1)[:n] = cl
            dst[s, base:base + k].reshape(-1)[:n] = dl
            wea[s, base:base + k].reshape(-1)[:n] = wl
            base += k
    return [int(x) for x in ncw], src, dst, wea


def _edge_we(e, n):
    row, col = np.asarray(e[0], np.int64), np.asarray(e[1], np.int64)
    deg = np.bincount(row, minlength=n).astype(np.float32)
    dis = np.where(deg > 0, 1.0 / np.sqrt(np.maximum(deg, 1.0)), 0.0).astype(np.float32)
    return row, col, -(dis[row] * dis[col]).astype(np.float32)


def _sub_edges(row, col, we, pool_idx):
    order = np.argsort(row, kind="stable")
    row, col, we = row[order], col[order], we[order]
    starts = np.searchsorted(row, pool_idx, side="left")
    ends = np.searchsorted(row, pool_idx, side="right")
    nr, ncl, nw = [], [], []
    for i in range(len(pool_idx)):
        s, e = starts[i], ends[i]
        if e > s:
            nr.append(np.full(e - s, i, np.int64))
            ncl.append(col[s:e])
            nw.append(we[s:e])
    return np.concatenate(nr), np.concatenate(ncl), np.concatenate(nw)


def _dense_s(row, col, we, n):
    s = np.zeros((n, n), np.float32)
    np.add.at(s, (row, col), we)
    return s


def _tile_w(w, pack):
    """[K, M] -> [K//(128*pack) * 128, pack*M]: pack K-blocks side by side."""
    k, m = w.shape
    nb = k // 128
    t = w.reshape(nb // pack, pack, 128, m).transpose(0, 2, 1, 3)
    return np.ascontiguousarray(t.reshape((nb // pack) * 128, pack * m))


def _host_prep(inputs):
    d = {k: np.asarray(v) for k, v in inputs.items()}
    x = d["x"].astype(np.float32)
    l0 = np.asarray(d["l0"], np.int64)
    l1 = np.asarray(d["l1"], np.int64)
    l2 = np.asarray(d["l2"], np.int64)

    X0 = np.ascontiguousarray(x.transpose(1, 0, 2).reshape(N0, 96))
    X0p = np.zeros((N0, 128), np.float32)
    X0p[:, :96] = X0
    X0l0T = np.ascontiguousarray(X0[l0].T)  # [96, 4096]

    r0, c0, w0 = _edge_we(d["e0"], N0)
    ncw_p1, src_p1, dst_p1, we_p1 = _prep_prop(r0, c0, w0, N0, NCORES)
    r0s, c0s, w0s = _sub_edges(r0, c0, w0, l0)
    ncw_p2, src_p2, dst_p2, we_p2 = _prep_prop(r0s, c0s, w0s, N1, NCORES)

    r1, c1, w1 = _edge_we(d["e1"], N1)
    ncw_q1, src_q1, dst_q1, we_q1 = _prep_prop(r1, c1, w1, N1, 1)
    r1s, c1s, w1s = _sub_edges(r1, c1, w1, l1)
    ncw_q2, src_q2, dst_q2, we_q2 = _prep_prop(r1s, c1s, w1s, N2, 1)

    r2, c2, w2 = _edge_we(d["e2"], N2)
    S2 = _dense_s(r2, c2, w2, N2)
    S2T = _tile_w(np.ascontiguousarray(S2.T), 8)       # [128, 8192]
    S2l2T = _tile_w(np.ascontiguousarray(S2[l2].T), 8)  # [128, 1024]
    P_l2 = np.zeros((N2, 128), np.float32)
    P_l2[l2, np.arange(128)] = 1.0
    P_l2 = _tile_w(P_l2, 8)                             # [128, 1024]

    r3, c3, w3 = _edge_we(d["e3"], N3)
    S3T = np.ascontiguousarray(_dense_s(r3, c3, w3, N3).T)

    def wmod(W):
        return W[0] - W[2], W[1], 2.0 * W[2]

    Wm1 = wmod(d["Wc1"].astype(np.float32))
    Wm = [wmod(d[f"Wc{i}"].astype(np.float32)) for i in (2, 3, 4, 5)]
    eye4 = np.eye(4, dtype=np.float32)

    per_core = []
    for k in range(NCORES):
        m = {}
        m["X0"] = X0p
        m["X0l0T"] = X0l0T
        m["iota"] = np.tile(np.arange(128, dtype=np.float32), (128, 1))
        m["epsv"] = np.full((128, 1), EPS, np.float32)
        m["l0_idx"] = np.ascontiguousarray(
            np.tile(l0.astype(np.int16).reshape(-1, 16).T, (8, 1)))
        m["l1_idx"] = np.ascontiguousarray(
            np.tile(l1.astype(np.int16).reshape(-1, 16).T, (8, 1)))
        for pref, (src, dst, wea) in (
            ("p1", (src_p1[k], dst_p1[k], we_p1[k])),
            ("p2", (src_p2[k], dst_p2[k], we_p2[k])),
            ("q1", (src_q1[0], dst_q1[0], we_q1[0])),
            ("q2", (src_q2[0], dst_q2[0], we_q2[0])),
        ):
            flat = src.reshape(-1).astype(np.int16)
            m[pref + "_srcw"] = np.ascontiguousarray(
                np.tile(flat.reshape(-1, 16).T, (8, 1)))
            m[pref + "_dst"] = np.ascontiguousarray(dst.transpose(1, 0))
            m[pref + "_we"] = np.ascontiguousarray(wea.transpose(1, 0))
        m["S2T"] = S2T
        m["S2l2T"] = S2l2T
        m["P_l2"] = P_l2
        m["S3T"] = S3T
        for t in range(3):
            bw = np.zeros((96, 128), np.float32)
            for j in range(4):
                bg = 4 * k + j
                bw[3 * bg:3 * bg + 3, 32 * j:32 * j + 32] = Wm1[t]
            m[f"bigw0_{t}"] = bw
        for lev in range(4):
            for t in range(3):
                m[f"bigw{lev + 1}_{t}"] = np.kron(eye4, Wm[lev][t])
        for lev, nm in ((1, "b1"), (2, "b2"), (3, "b3"), (4, "b4"), (5, "b5")):
            m[f"bias{lev}"] = np.tile(d[nm].astype(np.float32), 4).reshape(128, 1)
        for li in (6, 7, 8):
            W = d[f"W{li}"].astype(np.float32)[:, 512 * k:512 * k + 512]
            m[f"w{li}"] = _tile_w(W, 8)  # [512, 4096]
            m[f"g{li}"] = np.ascontiguousarray(
                d[f"g{li}"].astype(np.float32)[512 * k:512 * k + 512].reshape(4, 128).T)
            m[f"be{li}"] = np.ascontiguousarray(
                d[f"be{li}"].astype(np.float32)[512 * k:512 * k + 512].reshape(4, 128).T)
        m["w9"] = _tile_w(d["W9"].astype(np.float32)[512 * k:512 * k + 512], 4)  # [128, 512]
        per_core.append(m)

    meta = {"p1": ncw_p1, "p2": ncw_p2, "q1": ncw_q1, "q2": ncw_q2}
    return per_core, meta


# ---------------------------------------------------------------- device program
def _build_nc(meta, shapes):
    nc = bacc.Bacc("TRN2", target_bir_lowering=False, debug=False, num_devices=NCORES)
    ein = {}
    for name, arr in shapes.items():
        dt = {np.dtype(np.int32): I32, np.dtype(np.int16): I16}.get(arr.dtype, F32)
        ein[name] = nc.dram_tensor(name, list(arr.shape), dt, kind="ExternalInput")
    out_mu = nc.dram_tensor("mu", [128, 32], F32, kind="ExternalOutput")

    tx1_loc = nc.dram_tensor("tx1_loc", [N0 // 8, 128], F32)
    tx1_all = nc.dram_tensor("tx1_all", [N0, 128], F32)
    p2t_loc = nc.dram_tensor("p2t_loc", [96, 512], F32)
    p2t_all = nc.dram_tensor("p2t_all", [8 * 96, 512], F32)
    z1_dram = nc.dram_tensor("z1_dram", [N1, 128], F32)
    t1l1_dram = nc.dram_tensor("t1l1_dram", [N1, 128], F32)
    x6_loc = nc.dram_tensor("x6_loc", [4096, 4], F32)
    x6_all = nc.dram_tensor("x6_all", [8 * 4096, 4], F32)
    h6_loc = nc.dram_tensor("h6_loc", [512, 32], F32)
    h6_all = nc.dram_tensor("h6_all", [4096, 32], F32)
    h7_loc = nc.dram_tensor("h7_loc", [512, 32], F32)
    h7_all = nc.dram_tensor("h7_all", [4096, 32], F32)
    mu_loc = nc.dram_tensor("mu_loc", [128, 32], F32)
    mu_all = nc.dram_tensor("mu_all", [8 * 128, 32], F32)

    with tile.TileContext(nc) as tc:
        with (
            tc.tile_pool(name="const", bufs=1) as cpool,
            tc.tile_pool(name="big", bufs=1) as bigpool,
            tc.tile_pool(name="work", bufs=3) as wpool,
            tc.tile_pool(name="wload", bufs=2) as wlpool,
            tc.tile_pool(name="psA", bufs=3, space="PSUM") as ppool,
            tc.tile_pool(name="psB", bufs=1, space="PSUM") as apool,
        ):
            ident = cpool.tile([128, 128], F32, tag="ident", name="ident")
            make_identity(nc, ident[:])
            iota_t = cpool.tile([128, 128], F32, tag="iota", name="iota")
            nc.sync.dma_start(out=iota_t[:], in_=ein["iota"][:, :])
            eps_t = cpool.tile([128, 1], F32, tag="epsv", name="epsv")
            nc.sync.dma_start(out=eps_t[:], in_=ein["epsv"][:, :])

            def load_const(name):
                t = cpool.tile(list(shapes[name].shape), F32, tag=name)
                nc.sync.dma_start(out=t[:], in_=ein[name][:, :])
                return t

            def load_chunk_arrs(pref, C):
                s = cpool.tile([128, C * 8], I16, tag=pref + "s", name=pref + "s")
                dd = cpool.tile([128, C], F32, tag=pref + "d", name=pref + "d")
                w = cpool.tile([128, C], F32, tag=pref + "w", name=pref + "w")
                nc.sync.dma_start(out=s[:], in_=ein[pref + "_srcw"][:, :])
                nc.sync.dma_start(out=dd[:], in_=ein[pref + "_dst"][:, :])
                nc.sync.dma_start(out=w[:], in_=ein[pref + "_we"][:, :])
                return s, dd, w

            GRP = 16

            def grp_gather(idx_sb, g0, gc, gather_src):
                zb = wpool.tile([128, GRP * 128], F32, tag="zb", name="zb", bufs=3)
                nc.gpsimd.dma_gather(
                    out_ap=zb[:, :gc * 128].rearrange("p (c e) -> p c e", e=128),
                    in_ap=gather_src[:, :],
                    idxs_ap=idx_sb[:, g0 * 8:(g0 + gc) * 8],
                    num_idxs=gc * 128, num_idxs_reg=gc * 128, elem_size=128,
                    single_packet=False)
                return zb

            def mk_sel(eng, dst_ap, we_ap):
                sel = wpool.tile([128, 128], F32, tag="sel", name="sel")
                eng.tensor_scalar(out=sel[:], in0=iota_t[:], scalar1=dst_ap,
                                  scalar2=we_ap, op0=ALU.is_equal, op1=ALU.mult)
                return sel

            def prop_nodemajor(ncw, pref, gather_src, D, evac):
                C = sum(ncw)
                s, dd, w = load_chunk_arrs(pref, C)
                zbs = {}
                for g0 in range(0, C, GRP):
                    gc = min(GRP, C - g0)
                    zbs[g0] = grp_gather(s, g0, gc, gather_src)
                base = 0
                for wi, nch in enumerate(ncw):
                    ps = ppool.tile([128, 512], F32, tag="ps", name="ps")
                    for c in range(nch):
                        cc = base + c
                        zb = zbs[(cc // GRP) * GRP]
                        lo = (cc % GRP) * 128
                        sel = mk_sel(nc.vector, dd[:, cc:cc + 1], w[:, cc:cc + 1])
                        nc.tensor.matmul(out=ps[:, :D], lhsT=sel[:],
                                         rhs=zb[:, lo:lo + D],
                                         start=(c == 0), stop=(c == nch - 1))
                    evac(wi, ps[:, :D])
                    base += nch

            def transp(src_ap, dst_ap):
                p, f = src_ap.shape
                ps = ppool.tile([128, 512], F32, tag="ps", name="ps")
                nc.tensor.transpose(out=ps[:f, :p], in_=src_ap, identity=ident[:])
                nc.scalar.activation(out=dst_ap, in_=ps[:f, :p], func=AF.Copy)

            def gather_T(idx_t, chunks, gather_src, D, outT):
                chunks = list(chunks)
                zb = grp_gather(idx_t, chunks[0], len(chunks), gather_src)
                for ci in range(len(chunks)):
                    transp(zb[:, ci * 128:ci * 128 + D],
                           outT[:, ci * 128:(ci + 1) * 128])

            def einsum_win(bigw, taps, Din, width, out_ap, func, bias_ap):
                ps = ppool.tile([128, 512], F32, tag="ps", name="ps")
                for t in range(3):
                    nc.tensor.matmul(out=ps[:, :width], lhsT=bigw[t][:Din, :],
                                     rhs=taps[t], start=(t == 0), stop=(t == 2))
                f2 = AF.Identity if func == AF.Copy else func
                nc.scalar.activation(out=out_ap, in_=ps[:, :width], func=f2, bias=bias_ap)

            # ================= LEVEL 0 =================
            with nc.named_scope("l0_prop1"):
                def evac_p1(wi, ps_ap):
                    t = wpool.tile([128, 96], F32, tag="ev", name="ev", bufs=6)
                    nc.scalar.activation(out=t[:], in_=ps_ap, func=AF.Copy)
                    nc.sync.dma_start(out=tx1_loc[wi * 128:(wi + 1) * 128, :96], in_=t[:])
                prop_nodemajor(meta["p1"], "p1", ein["X0"], 96, evac_p1)
            with nc.named_scope("ag1"):
                nc.gpsimd.collective_compute(
                    "AllGather", ALU.bypass, replica_groups=RG,
                    ins=[tx1_loc.ap().opt()], outs=[tx1_all.ap().opt()])

            with nc.named_scope("l0_prop2"):
                C2 = sum(meta["p2"])
                s2c, d2c, w2c = load_chunk_arrs("p2", C2)
                zbs2 = {}
                for g0 in range(0, C2, GRP):
                    gc = min(GRP, C2 - g0)
                    zbs2[g0] = grp_gather(s2c, g0, gc, tx1_all)
                p2t_sb = bigpool.tile([96, 512], F32, tag="p2t_sb", name="p2t_sb")
                base = 0
                for wi, nch in enumerate(meta["p2"]):
                    ps = ppool.tile([128, 512], F32, tag="ps", name="ps")
                    for c in range(nch):
                        cc = base + c
                        zb = zbs2[(cc // GRP) * GRP]
                        lo = (cc % GRP) * 128
                        sel = mk_sel(nc.vector, d2c[:, cc:cc + 1], w2c[:, cc:cc + 1])
                        nc.tensor.matmul(out=ps[:96, :128],
                                         lhsT=zb[:, lo:lo + 96], rhs=sel[:],
                                         start=(c == 0), stop=(c == nch - 1))
                    nc.scalar.activation(out=p2t_sb[:, wi * 128:(wi + 1) * 128],
                                         in_=ps[:96, :128], func=AF.Copy)
                    base += nch
                nc.sync.dma_start(out=p2t_loc[:, :], in_=p2t_sb[:])
            with nc.named_scope("ag2"):
                nc.gpsimd.collective_compute(
                    "AllGather", ALU.bypass, replica_groups=RG,
                    ins=[p2t_loc.ap().opt()], outs=[p2t_all.ap().opt()])

            with nc.named_scope("l0_einsum"):
                l0i = cpool.tile([128, 32 * 8], I16, tag="l0i", name="l0i")
                nc.sync.dma_start(out=l0i[:], in_=ein["l0_idx"][:, :])
                bw0 = [load_const(f"bigw0_{t}") for t in range(3)]
                bias1 = load_const("bias1")
                for w in range(8):
                    g0w = wpool.tile([96, 512], F32, tag="g0w", name="g0w")
                    nc.sync.dma_start(out=g0w[:], in_=ein["X0l0T"][:, 512 * w:512 * (w + 1)])
                    g1w = wpool.tile([96, 512], F32, tag="g1w", name="g1w")
                    gather_T(l0i, range(4 * w, 4 * w + 4), tx1_all, 96, g1w)
                    p2w = wpool.tile([96, 512], F32, tag="p2w", name="p2w")
                    nc.sync.dma_start(out=p2w[:], in_=p2t_all[96 * w:96 * (w + 1), :])
                    z1Tw = wpool.tile([128, 512], F32, tag="z1Tw", name="z1Tw")
                    einsum_win(bw0, [g0w[:], g1w[:], p2w[:]], 96, 512,
                               z1Tw[:], AF.Copy, bias1[:, 0:1])
                    for c in range(4):
                        t = wpool.tile([128, 128], F32, tag="z1nc", name="z1nc")
                        transp(z1Tw[:, c * 128:(c + 1) * 128], t[:])
                        r = w * 512 + c * 128
                        nc.sync.dma_start(out=z1_dram[r:r + 128, :], in_=t[:])

            # ================= LEVEL 1 =================
            with nc.named_scope("l1_prop1"):
                def evac_q1(wi, ps_ap):
                    t = wpool.tile([128, 128], F32, tag="ev", name="ev", bufs=6)
                    nc.scalar.activation(out=t[:], in_=ps_ap, func=AF.Copy)
                    nc.sync.dma_start(out=t1l1_dram[wi * 128:(wi + 1) * 128, :], in_=t[:])
                prop_nodemajor(meta["q1"], "q1", z1_dram, 128, evac_q1)

            p2n_l1 = bigpool.tile([128, 8 * 128], F32, tag="p2n_l1", name="p2n_l1")
            with nc.named_scope("l1_prop2"):
                def evac_q2(wi, ps_ap):
                    nc.scalar.activation(out=p2n_l1[:, wi * 128:(wi + 1) * 128],
                                         in_=ps_ap, func=AF.Copy)
                prop_nodemajor(meta["q2"], "q2", t1l1_dram, 128, evac_q2)

            z2n = bigpool.tile([128, 8 * 128], F32, tag="z2n", name="z2n")
            with nc.named_scope("l1_einsum"):
                l1i = cpool.tile([128, 8 * 8], I16, tag="l1i", name="l1i")
                nc.sync.dma_start(out=l1i[:], in_=ein["l1_idx"][:, :])
                z1l1T = bigpool.tile([128, 1024], F32, tag="z1l1T", name="z1l1T")
                gather_T(l1i, range(8), z1_dram, 128, z1l1T)
                t1l1T = bigpool.tile([128, 1024], F32, tag="t1l1T", name="t1l1T")
                gather_T(l1i, range(8), t1l1_dram, 128, t1l1T)
                p2l1T = bigpool.tile([128, 1024], F32, tag="p2l1T", name="p2l1T")
                for c in range(8):
                    transp(p2n_l1[:, c * 128:(c + 1) * 128], p2l1T[:, c * 128:(c + 1) * 128])
                bw1 = [load_const(f"bigw1_{t}") for t in range(3)]
                bias2 = load_const("bias2")
                z2T = bigpool.tile([128, 1024], F32, tag="z2T", name="z2T")
                for w in range(2):
                    einsum_win(bw1, [z1l1T[:, 512 * w:512 * (w + 1)],
                                     t1l1T[:, 512 * w:512 * (w + 1)],
                                     p2l1T[:, 512 * w:512 * (w + 1)]],
                               128, 512, z2T[:, 512 * w:512 * (w + 1)], AF.Tanh, bias2[:, 0:1])
                for c in range(8):
                    transp(z2T[:, c * 128:(c + 1) * 128], z2n[:, c * 128:(c + 1) * 128])

            # ================= LEVEL 2 (dense) =================
            with nc.named_scope("l2"):
                t1_l2 = bigpool.tile([128, 8 * 128], F32, tag="t1_l2", name="t1_l2")
                for half in range(2):
                    s2t = wlpool.tile([128, 4096], F32, tag="wld", name="wld")
                    nc.sync.dma_start(out=s2t[:], in_=ein["S2T"][:, 4096 * half:4096 * (half + 1)])
                    for dc in range(8):
                        ps = ppool.tile([128, 512], F32, tag="ps", name="ps")
                        for kk in range(4):
                            kc = half * 4 + kk
                            nc.tensor.matmul(
                                out=ps[:, :128],
                                lhsT=s2t[:, kk * 1024 + dc * 128: kk * 1024 + dc * 128 + 128],
                                rhs=z2n[:, kc * 128:(kc + 1) * 128],
                                start=(kk == 0), stop=(kk == 3))
                        if half == 0:
                            nc.scalar.activation(out=t1_l2[:, dc * 128:(dc + 1) * 128],
                                                 in_=ps[:, :128], func=AF.Copy)
                        else:
                            nc.vector.tensor_add(t1_l2[:, dc * 128:(dc + 1) * 128],
                                                 t1_l2[:, dc * 128:(dc + 1) * 128],
                                                 ps[:, :128])
                s2l2 = cpool.tile([128, 1024], F32, tag="s2l2", name="s2l2")
                nc.sync.dma_start(out=s2l2[:], in_=ein["S2l2T"][:, :])
                ps = ppool.tile([128, 512], F32, tag="ps", name="ps")
                for kc in range(8):
                    nc.tensor.matmul(out=ps[:, :128], lhsT=s2l2[:, kc * 128:(kc + 1) * 128],
                                     rhs=t1_l2[:, kc * 128:(kc + 1) * 128],
                                     start=(kc == 0), stop=(kc == 7))
                p2n_l2 = wpool.tile([128, 128], F32, tag="p2n_l2", name="p2n_l2")
                nc.scalar.activation(out=p2n_l2[:], in_=ps[:, :128], func=AF.Copy)
                pl2 = cpool.tile([128, 1024], F32, tag="pl2", name="pl2")
                nc.sync.dma_start(out=pl2[:], in_=ein["P_l2"][:, :])
                z2l2T = wpool.tile([128, 128], F32, tag="z2l2T", name="z2l2T")
                psg = ppool.tile([128, 512], F32, tag="ps", name="ps")
                for kc in range(8):
                    nc.tensor.matmul(out=psg[:, :128], lhsT=z2n[:, kc * 128:(kc + 1) * 128],
                                     rhs=pl2[:, kc * 128:(kc + 1) * 128],
                                     start=(kc == 0), stop=(kc == 7))
                nc.scalar.activation(out=z2l2T[:], in_=psg[:, :128], func=AF.Copy)
                t1l2T = wpool.tile([128, 128], F32, tag="t1l2T", name="t1l2T")
                psg2 = ppool.tile([128, 512], F32, tag="ps", name="ps")
                for kc in range(8):
                    nc.tensor.matmul(out=psg2[:, :128], lhsT=t1_l2[:, kc * 128:(kc + 1) * 128],
                                     rhs=pl2[:, kc * 128:(kc + 1) * 128],
                                     start=(kc == 0), stop=(kc == 7))
                nc.scalar.activation(out=t1l2T[:], in_=psg2[:, :128], func=AF.Copy)
                p2l2T = wpool.tile([128, 128], F32, tag="p2l2T", name="p2l2T")
                transp(p2n_l2[:], p2l2T[:])
                bw2 = [load_const(f"bigw2_{t}") for t in range(3)]
                bias3 = load_const("bias3")
                z3T = wpool.tile([128, 128], F32, tag="z3T", name="z3T")
                einsum_win(bw2, [z2l2T[:], t1l2T[:], p2l2T[:]], 128, 128,
                           z3T[:], AF.Tanh, bias3[:, 0:1])
                z3n = wpool.tile([128, 128], F32, tag="z3n", name="z3n")
                transp(z3T[:], z3n[:])

            # ================= LEVEL 3 =================
            with nc.named_scope("l3"):
                s3t = cpool.tile([128, 128], F32, tag="s3t", name="s3t")
                nc.sync.dma_start(out=s3t[:], in_=ein["S3T"][:, :])
                bias4 = load_const("bias4")
                bias5 = load_const("bias5")

                def conv_l3(zn, zT, bw_pref, bias_t, func, keep):
                    t1T = wpool.tile([128, 128], F32, tag=keep + "t1T", name=keep + "t1T")
                    ps = ppool.tile([128, 512], F32, tag="ps", name="ps")
                    nc.tensor.matmul(out=ps[:, :128], lhsT=zn, rhs=s3t[:], start=True, stop=True)
                    nc.scalar.activation(out=t1T[:], in_=ps[:, :128], func=AF.Copy)
                    t1n_ = wpool.tile([128, 128], F32, tag=keep + "t1n", name=keep + "t1n")
                    transp(t1T[:], t1n_[:])
                    p2T_ = wpool.tile([128, 128], F32, tag=keep + "p2T", name=keep + "p2T")
                    ps2 = ppool.tile([128, 512], F32, tag="ps", name="ps")
                    nc.tensor.matmul(out=ps2[:, :128], lhsT=t1n_[:], rhs=s3t[:], start=True, stop=True)
                    nc.scalar.activation(out=p2T_[:], in_=ps2[:, :128], func=AF.Copy)
                    bw = [load_const(f"{bw_pref}_{t}") for t in range(3)]
                    outT = wpool.tile([128, 128], F32, tag=keep + "oT", name=keep + "oT")
                    einsum_win(bw, [zT, t1T[:], p2T_[:]], 128, 128, outT[:], func, bias_t[:, 0:1])
                    outn = wpool.tile([128, 128], F32, tag=keep + "on", name=keep + "on")
                    transp(outT[:], outn[:])
                    return outn, outT

                z4n, z4T = conv_l3(z3n[:], z3T[:], "bigw3", bias4, AF.Tanh, "c4")
                o5n, o5T = conv_l3(z4n[:], z4T[:], "bigw4", bias5, AF.Copy, "c5")

            # ================= MLP input assembly =================
            with nc.named_scope("mlp_in"):
                for j in range(4):
                    ap_out = x6_loc.ap()[:, j:j + 1].rearrange("(n h) o -> n (h o)", h=32)
                    nc.sync.dma_start(out=ap_out, in_=o5n[:, 32 * j:32 * j + 32])
                nc.gpsimd.collective_compute(
                    "AllGather", ALU.bypass, replica_groups=RG,
                    ins=[x6_loc.ap().opt()], outs=[x6_all.ap().opt()])

            # ================= MLP =================
            def mlp_layer(nm, src_sb, out_sb):
                g_t = load_const("g" + nm[1])
                be_t = load_const("be" + nm[1])
                pss = [apool.tile([128, 32], F32, tag=f"acc{m}", name=f"acc{m}") for m in range(4)]
                for i in range(4):
                    wt = wlpool.tile([128, 4096], F32, tag="wld", name="wld")
                    nc.sync.dma_start(out=wt[:], in_=ein[nm][128 * i:128 * (i + 1), :])
                    for a in range(8):
                        kc = i * 8 + a
                        for mm in range(4):
                            nc.tensor.matmul(
                                out=pss[mm][:],
                                lhsT=wt[:, a * 512 + mm * 128: a * 512 + mm * 128 + 128],
                                rhs=src_sb[:, 32 * kc:32 * kc + 32],
                                start=(kc == 0), stop=(kc == 31))
                for mm in range(4):
                    t = wpool.tile([128, 32], F32, tag="b_t", name="b_t")
                    nc.vector.tensor_copy(t[:], pss[mm][:])
                    s1 = wpool.tile([128, 1], F32, tag="b_s1", name="b_s1")
                    nc.vector.tensor_reduce(out=s1[:], in_=t[:], axis=AX.X, op=ALU.add)
                    mu_ = wpool.tile([128, 1], F32, tag="b_mu", name="b_mu")
                    nc.vector.tensor_scalar_mul(mu_[:], s1[:], 1.0 / 32.0)
                    sq = wpool.tile([128, 32], F32, tag="b_sq", name="b_sq")
                    nc.vector.tensor_mul(sq[:], t[:], t[:])
                    s2_ = wpool.tile([128, 1], F32, tag="b_s2", name="b_s2")
                    nc.vector.tensor_reduce(out=s2_[:], in_=sq[:], axis=AX.X, op=ALU.add)
                    var = wpool.tile([128, 1], F32, tag="b_var", name="b_var")
                    nc.vector.scalar_tensor_tensor(out=var[:], in0=mu_[:], scalar=-1.0,
                                                   in1=mu_[:], op0=ALU.mult, op1=ALU.mult)
                    nc.vector.scalar_tensor_tensor(out=var[:], in0=s2_[:], scalar=1.0 / 32.0,
                                                   in1=var[:], op0=ALU.mult, op1=ALU.add)
                    sd = wpool.tile([128, 1], F32, tag="b_sd", name="b_sd")
                    nc.scalar.activation(out=sd[:], in_=var[:], func=AF.Sqrt, bias=eps_t[:, 0:1])
                    rs = wpool.tile([128, 1], F32, tag="b_rs", name="b_rs")
                    nc.vector.reciprocal(rs[:], sd[:])
                    a_ = wpool.tile([128, 1], F32, tag="b_a", name="b_a")
                    nc.vector.tensor_mul(a_[:], rs[:], g_t[:, mm:mm + 1])
                    sh = wpool.tile([128, 1], F32, tag="b_sh", name="b_sh")
                    nc.vector.scalar_tensor_tensor(out=sh[:], in0=mu_[:], scalar=-1.0,
                                                   in1=a_[:], op0=ALU.mult, op1=ALU.mult)
                    nc.vector.tensor_add(sh[:], sh[:], be_t[:, mm:mm + 1])
                    nc.scalar.activation(out=out_sb[:, 32 * mm:32 * mm + 32], in_=t[:],
                                         func=AF.Relu, scale=a_[:, 0:1], bias=sh[:, 0:1])

            with nc.named_scope("mlp6"):
                x6T = bigpool.tile([128, 1024], F32, tag="x6T", name="x6T")
                for kk in range(8):
                    nc.sync.dma_start(
                        out=x6T[:].rearrange("p (c r) -> p c r", r=32)[:, :, 4 * kk:4 * kk + 4],
                        in_=x6_all[4096 * kk:4096 * (kk + 1), :].rearrange(
                            "(c p) j -> p c j", p=128))
                h6 = bigpool.tile([128, 128], F32, tag="h6sb", name="h6sb")
                mlp_layer("w6", x6T, h6)
                nc.sync.dma_start(out=h6_loc.ap().rearrange("(m p) b -> p m b", p=128),
                                  in_=h6[:].rearrange("p (m b) -> p m b", b=32))
                nc.gpsimd.collective_compute(
                    "AllGather", ALU.bypass, replica_groups=RG,
                    ins=[h6_loc.ap().opt()], outs=[h6_all.ap().opt()])
            with nc.named_scope("mlp7"):
                x7T = bigpool.tile([128, 1024], F32, tag="x7T", name="x7T")
                nc.sync.dma_start(out=x7T[:].rearrange("p (c b) -> p c b", b=32),
                                  in_=h6_all[:, :].rearrange("(c p) b -> p c b", p=128))
                h7 = bigpool.tile([128, 128], F32, tag="h7sb", name="h7sb")
                mlp_layer("w7", x7T, h7)
                nc.sync.dma_start(out=h7_loc.ap().rearrange("(m p) b -> p m b", p=128),
                                  in_=h7[:].rearrange("p (m b) -> p m b", b=32))
                nc.gpsimd.collective_compute(
                    "AllGather", ALU.bypass, replica_groups=RG,
                    ins=[h7_loc.ap().opt()], outs=[h7_all.ap().opt()])
            with nc.named_scope("mlp8"):
                x8T = bigpool.tile([128, 1024], F32, tag="x8T", name="x8T")
                nc.sync.dma_start(out=x8T[:].rearrange("p (c b) -> p c b", b=32),
                                  in_=h7_all[:, :].rearrange("(c p) b -> p c b", p=128))
                h8 = bigpool.tile([128, 128], F32, tag="h8sb", name="h8sb")
                mlp_layer("w8", x8T, h8)

            with nc.named_scope("mlp9"):
                w9t = cpool.tile([128, 512], F32, tag="w9t", name="w9t")
                nc.sync.dma_start(out=w9t[:], in_=ein["w9"][:, :])
                ps9 = apool.tile([128, 32], F32, tag="acc0", name="acc0")
                for kc in range(4):
                    nc.tensor.matmul(out=ps9[:], lhsT=w9t[:, kc * 128:(kc + 1) * 128],
                                     rhs=h8[:, 32 * kc:32 * kc + 32],
                                     start=(kc == 0), stop=(kc == 3))
                mu_sb = wpool.tile([128, 32], F32, tag="mu_sb", name="mu_sb")
                nc.scalar.activation(out=mu_sb[:], in_=ps9[:], func=AF.Copy)
                nc.sync.dma_start(out=mu_loc[:, :], in_=mu_sb[:])
                nc.gpsimd.collective_compute(
                    "AllGather", ALU.bypass, replica_groups=RG,
                    ins=[mu_loc.ap().opt()], outs=[mu_all.ap().opt()])
                tot = wpool.tile([128, 32], F32, tag="f_tot", name="f_tot")
                nc.sync.dma_start(out=tot[:], in_=mu_all[0:128, :])
                for k in range(1, 8):
                    pk = wpool.tile([128, 32], F32, tag="f_pk", name="f_pk")
                    nc.sync.dma_start(out=pk[:], in_=mu_all[k * 128:(k + 1) * 128, :])
                    nc.vector.tensor_add(tot[:], tot[:], pk[:])
                s1 = wpool.tile([128, 1], F32, tag="f_s1", name="f_s1")
                nc.vector.tensor_reduce(out=s1[:], in_=tot[:], axis=AX.X, op=ALU.add)
                mu_ = wpool.tile([128, 1], F32, tag="f_mu", name="f_mu")
                nc.vector.tensor_scalar_mul(mu_[:], s1[:], 1.0 / 32.0)
                sq = wpool.tile([128, 32], F32, tag="f_sq", name="f_sq")
                nc.vector.tensor_mul(sq[:], tot[:], tot[:])
                s2_ = wpool.tile([128, 1], F32, tag="f_s2", name="f_s2")
                nc.vector.tensor_reduce(out=s2_[:], in_=sq[:], axis=AX.X, op=ALU.add)
                var = wpool.tile([128, 1], F32, tag="f_var", name="f_var")
                nc.vector.scalar_tensor_tensor(out=var[:], in0=mu_[:], scalar=-1.0,
                                               in1=mu_[:], op0=ALU.mult, op1=ALU.mult)
                nc.vector.scalar_tensor_tensor(out=var[:], in0=s2_[:], scalar=1.0 / 32.0,
                                               in1=var[:], op0=ALU.mult, op1=ALU.add)
                sdf = wpool.tile([128, 1], F32, tag="f_sd", name="f_sd")
                nc.scalar.activation(out=sdf[:], in_=var[:], func=AF.Sqrt, bias=eps_t[:, 0:1])
                rs = wpool.tile([128, 1], F32, tag="f_rs", name="f_rs")
                nc.vector.reciprocal(rs[:], sdf[:])
                neg = wpool.tile([128, 1], F32, tag="f_neg", name="f_neg")
                nc.vector.scalar_tensor_tensor(out=neg[:], in0=mu_[:], scalar=-1.0,
                                               in1=rs[:], op0=ALU.mult, op1=ALU.mult)
                outt = wpool.tile([128, 32], F32, tag="f_out", name="f_out")
                nc.scalar.activation(out=outt[:], in_=tot[:], func=AF.Identity,
                                     scale=rs[:, 0:1], bias=neg[:, 0:1])
                nc.sync.dma_start(out=out_mu[:, :], in_=outt[:])

    nc.compile()
    return nc


# ---------------------------------------------------------------- entry point
def kernel(**inputs) -> np.ndarray:
    per_core, meta = _host_prep(inputs)
    if "prog" not in _CACHE:
        _CACHE["prog"] = _build_nc(meta, per_core[0])
    nc = _CACHE["prog"]
    res = bass_utils.run_bass_kernel_spmd(nc, per_core, core_ids=list(range(NCORES)))
    return np.ascontiguousarray(res.results[0]["mu"].T)

